# revision 38
# baseline (speedup 1.0000x reference)
"""Bass/Trainium2 kernel for BestMatchDistance.

ref: sim[b,q,s] = sum_d q[b,d,q]*s[b,d,s]; out[b] = mean_q max_s sim.

Sharding: batch dim B=64 split across 8 cores (8 batches/core), pure data
parallel. Inputs are cast to bf16 on the host (full-rate PE, half DMA).

Per (batch, 128-query tile): the [128, 2048] sim row = 4 bf16 matmuls
(K=64, N=512) K-packed 2-up onto PE row-groups 0-63 / 64-127 (query data is
duplicated to both partition halves, support is split), so weight loads and
matmuls of the two groups overlap on the systolic array. The s-columns land
permuted across PSUM, which is irrelevant under a max-reduce.

PSUM per row (SCAN_MODE, the shipped path): two 2-bank tiles, double
buffered = 8 banks, each read by exactly ONE engine so its release waits on
exactly one consumer:
  - ScalarE copies its tile to SBUF bf16 in one contiguous instruction.
  - VectorE runs ONE tensor_tensor_scan: state=(psum[t] max state) max
    copy[t] -- a running max folding both 1024-wide streams at 2 elem/cycle,
    with the out AP broadcast to [128,1] so the final state IS the row max
    (no dump buffer, no tree).
Mean over queries = free-dim reduce_sum + ones-vector matmul over
partitions, scaled by 1/NQ.

Alternatives measured on this stack and rejected: ScalarE
activation+accum_out (LSE head) costs ~2.5-5us/instruction on HW;
tensor_tensor_reduce crashes NEFF execution (also in its canonical qr.py
form); GpSimd tensor_tensor fails NEFF lowering / tensor_reduce is
partition-axis only; non-bank-aligned ScalarE/VectorE PSUM splits serialize
(+340us); a single 4-bank row tile (both consumers on one tile) regresses
sim 196.7->242.9. SCAN_MODE=False falls back to the ScalarE-copy +
VectorE-tree design (sim 208.9 vs 183.6 for the scan layout).
"""

import numpy as np

B, D, NQ, NS = 64, 64, 2048, 2048
XW_CFG = 512  # PSUM A width (direct DVE reduce); rest goes via ACT copy
TREE_HB = 8  # rows per bf16-tree instruction batch (finer interleave: the
# per-batch DVE tree chain halves, so it stalls the next batch's direct
# reduces less in the engine FIFO; TimelineSim 209.6us vs 214.1 at HB=16)
TREE_MIN = 48  # smallest TT level width; tail reduce runs on this width
B_FIRST = True  # emit B (ACT-copied) matmuls before the A (DVE) matmul
POOL_L1 = False  # GpSimdE tree offload (left off: Pool TT fails NEFF lowering)
W_POOL = 0  # if >0: stop DVE tree at this width; GpSimdE reduce_max finishes
PAIR_A = True  # batch 2 rows' A-banks in one PSUM tile: one DVE reduce per
# pair halves the 120-cycle tensor_reduce overhead (pa bufs drops to 1)
SCAN_MODE = True  # full-row reduce via ONE tensor_tensor_scan per row:
# state = max(data0[t], state, data1[t]); data0 = B banks 1-2 (PSUM),
# data1 = ScalarE copies of {A bank, B bank 3} (SBUF bf16); out is a
# broadcast [128,1] AP so the final state IS the row max (no dump).
N_CORES = 8
BPC = B // N_CORES  # batches per core

_cache = {}


def _emit_body_scan(nc, mybir, q_d, s_d, o_d, ones, rall, pools, rep=0):
    f32 = mybir.dt.float32
    bf16 = mybir.dt.bfloat16
    fmax = mybir.AluOpType.max
    X = mybir.AxisListType.X
    qp, sp, pa, pb, bcp, trp, rp, finp = pools

    n_qt = NQ // 128
    XW = 512
    YW = NS - XW
    HNS = NS // 2

    for b in range(BPC):
        qt = qp.tile([128, NQ], bf16, tag="q", name=f"q{rep}_{b}")
        nc.sync.dma_start(out=qt[0:64, :], in_=q_d[b])
        nc.sync.dma_start(out=qt[64:128, :], in_=q_d[b])
        st = sp.tile([128, HNS], bf16, tag="s", name=f"s{rep}_{b}")
        nc.sync.dma_start(out=st[0:64, :], in_=s_d[b][:, 0:HNS])
        nc.sync.dma_start(out=st[64:128, :], in_=s_d[b][:, HNS:NS])

        rM = rp.tile([128, n_qt], f32, tag="rM", name=f"rM{rep}_{b}")
        for i in range(n_qt):
            # one 2-bank PSUM tile per consumer: Ct is read ONLY by
            # ScalarE's single contiguous copy, Vt ONLY by the scan --
            # each tile's release waits on exactly one engine
            Ct = pa.tile([128, 1024], f32, tag="A", name=f"A{rep}_{b}_{i}")
            Vt = pb.tile([128, 1024], f32, tag="B", name=f"B{rep}_{b}_{i}")
            lhs0 = qt[0:64, i * 128 : (i + 1) * 128]
            lhs1 = qt[64:128, i * 128 : (i + 1) * 128]
            # ScalarE's banks first (grp0/grp1 alternating so PE packs)
            nc.tensor.matmul(
                Ct[:, 0:512], lhsT=lhs0, rhs=st[0:64, 0:512],
                start=True, stop=True,
            )
            nc.tensor.matmul(
                Ct[:, 512:1024], lhsT=lhs1, rhs=st[64:128, 0:512],
                start=True, stop=True, tile_position=(64, 0),
            )
            nc.tensor.matmul(
                Vt[:, 0:512], lhsT=lhs0, rhs=st[0:64, 512:1024],
                start=True, stop=True,
            )
            nc.tensor.matmul(
                Vt[:, 512:1024], lhsT=lhs1, rhs=st[64:128, 512:1024],
                start=True, stop=True, tile_position=(64, 0),
            )
            sbc = bcp.tile([128, 1024], bf16, tag="sc", name=f"sc{rep}_{b}_{i}")
            nc.scalar.copy(out=sbc[:], in_=Ct[:])
            # One DVE scan folds the PSUM tile with the copies; the
            # broadcast out makes the final state land in rM[:, i].
            nc.vector.tensor_tensor_scan(
                out=rM[:, i : i + 1].broadcast_to((128, 1024)),
                data0=Vt[:], data1=sbc[:],
                initial=-1e30, op0=fmax, op1=fmax,
            )
        nc.vector.reduce_sum(rall[:, b : b + 1], rM[:], axis=X)

    pf = pa.tile([1, BPC], f32, tag="A", name=f"pf{rep}")
    nc.tensor.matmul(pf[:], lhsT=ones[:], rhs=rall[:], start=True, stop=True)
    ob = finp.tile([1, BPC], f32, tag="ob", name=f"ob{rep}")
    nc.scalar.mul(ob[:], pf[:], 1.0 / NQ)
    nc.sync.dma_start(out=o_d[:], in_=ob[:])


def _emit_body(nc, mybir, q_d, s_d, o_d, ones, rall, pools, rep=0, parts=31):
    DO_MM = parts & 1
    DO_RA = parts & 2
    DO_CP = parts & 4
    DO_TREE = parts & 8
    f32 = mybir.dt.float32
    bf16 = mybir.dt.bfloat16
    fmax = mybir.AluOpType.max
    X = mybir.AxisListType.X
    qp, sp, pa, pb, bcp, trp, rp, finp = pools

    n_qt = NQ // 128  # 16 q-tiles per batch
    HB = TREE_HB  # rows per tree batch
    XW = XW_CFG  # direct-reduce width (PSUM A)
    YW = NS - XW  # ACT-copied width (PSUM B), 2 banks
    HNS = NS // 2  # support cols per row-group

    if SCAN_MODE and DO_MM and DO_RA and DO_CP and DO_TREE:
        _emit_body_scan(nc, mybir, q_d, s_d, o_d, ones, rall, pools, rep)
        return

    # Deferred DVE work (tree chains, per-batch combines): emitted one
    # instruction per subsequent row so the engine FIFO interleaves them
    # with the direct A-reduces instead of bunching a multi-us chain that
    # delays PSUM releases (and thus PE).
    from collections import deque

    pending = deque()

    for b in range(BPC):
        qt = qp.tile([128, NQ], bf16, tag="q", name=f"q{rep}_{b}")
        nc.sync.dma_start(out=qt[0:64, :], in_=q_d[b])
        nc.sync.dma_start(out=qt[64:128, :], in_=q_d[b])
        st = sp.tile([128, HNS], bf16, tag="s", name=f"s{rep}_{b}")
        nc.sync.dma_start(out=st[0:64, :], in_=s_d[b][:, 0:HNS])
        nc.sync.dma_start(out=st[64:128, :], in_=s_d[b][:, HNS:NS])

        rA = None
        if XW > 0:
            rA = rp.tile(
                [128, n_qt, XW // 512], f32, tag="rA", name=f"rA{rep}_{b}"
            )
            if not DO_RA:
                nc.vector.memset(rA[:], 0.0)
        rB = rp.tile([128, n_qt], bf16, tag="rB", name=f"rB{rep}_{b}")
        if not (DO_TREE and DO_CP):
            nc.vector.memset(rB[:], 0.0)

        for h in range(n_qt // HB):
            bc = bcp.tile([128, HB, YW], bf16, tag="bc", name=f"bc{rep}_{b}_{h}")
            for r in range(HB):
                i = h * HB + r
                if pending:
                    pending.popleft()()
                if XW == 0:
                    A = None
                elif PAIR_A:
                    if i % 2 == 0:
                        Apair = pa.tile(
                            [128, 2, XW], f32, tag="A", name=f"A{rep}_{b}_{i}"
                        )
                    A = Apair[:, i % 2]
                else:
                    A = pa.tile([128, XW], f32, tag="A", name=f"A{rep}_{b}_{i}")
                Bt = pb.tile([128, YW], f32, tag="B", name=f"B{rep}_{b}_{i}")
                lhs0 = qt[0:64, i * 128 : (i + 1) * 128]
                lhs1 = qt[64:128, i * 128 : (i + 1) * 128]
                if DO_MM:
                    # 4 N=512 matmuls, K-packed: grp0 covers s-cols [0,HNS),
                    # grp1 covers [HNS,NS). Destinations fill A banks then B.
                    dsts = [
                        (A, j * 512) for j in range(XW // 512)
                    ] + [(Bt, j * 512) for j in range(YW // 512)]
                    if B_FIRST:
                        dsts = dsts[XW // 512 :] + dsts[: XW // 512]
                    for k4 in range(4):
                        grp = k4 % 2
                        sc = (k4 // 2) * 512
                        dst, off = dsts[k4]
                        if grp == 0:
                            nc.tensor.matmul(
                                dst[:, off : off + 512], lhsT=lhs0,
                                rhs=st[0:64, sc : sc + 512],
                                start=True, stop=True,
                            )
                        else:
                            nc.tensor.matmul(
                                dst[:, off : off + 512], lhsT=lhs1,
                                rhs=st[64:128, sc : sc + 512],
                                start=True, stop=True, tile_position=(64, 0),
                            )
                if DO_RA and PAIR_A:
                    if i % 2 == 1:
                        nc.vector.reduce_max(
                            rA[:, i - 1 : i + 1, 0:1], Apair[:], axis=X
                        )
                elif DO_RA:
                    for j in range(XW // 512):
                        nc.vector.reduce_max(
                            rA[:, i, j : j + 1],
                            A[:, j * 512 : (j + 1) * 512],
                            axis=X,
                        )
                if DO_CP:
                    nc.scalar.copy(out=bc[:, r], in_=Bt[:])

            if not (DO_TREE and DO_CP):
                continue
            # bf16 max tree over [128, HB, YW] -> [128, HB], deferred:
            # each level is queued and emitted before a later row's matmuls
            levels = []
            w = YW // 2
            lvl = 0
            while w >= TREE_MIN:
                levels.append((lvl, w))
                w //= 2
                lvl += 1

            def emit_level(lv=None, b=b, h=h, bc=bc, state={}):
                cur_t = state.get("cur", bc)
                lvl, w = lv
                nxt_t = trp.tile(
                    [128, HB, w], bf16, tag=f"t{lvl}", name=f"t{lvl}_{rep}_{b}_{h}"
                )
                nc.vector.tensor_tensor(
                    out=nxt_t[:], in0=cur_t[:, :, 0:w],
                    in1=cur_t[:, :, w : 2 * w], op=fmax,
                )
                state["cur"] = nxt_t
                return state

            state = {}
            for lv in levels:
                pending.append(
                    lambda lv=lv, st=state, b=b, h=h, bc=bc: emit_level(
                        lv, b, h, bc, st
                    )
                )

            def emit_red(st=state, b=b, h=h, bc=bc, rB=rB):
                cur_t = st.get("cur", bc)
                nc.vector.reduce_max(
                    rB[:, h * HB : (h + 1) * HB], cur_t[:], axis=X
                )

            pending.append(emit_red)

        # combine: per-q max over {A bank maxes, B tree maxes} (deferred)
        def emit_combine(b=b, rA=rA, rB=rB):
            nb = XW // 512
            cur = rB
            for j in range(nb):
                nxt = rp.tile(
                    [128, n_qt], f32, tag=f"rc{j}", name=f"rc{j}_{rep}_{b}"
                )
                nc.vector.tensor_tensor(
                    out=nxt[:], in0=cur[:], in1=rA[:, :, j], op=fmax
                )
                cur = nxt
            nc.vector.reduce_sum(rall[:, b : b + 1], cur[:], axis=X)

        if DO_TREE and DO_CP:
            pending.append(emit_combine)

    while pending:
        pending.popleft()()

    if XW == 0:
        pf = pb.tile([1, BPC], f32, tag="B", name=f"pf{rep}")
    else:
        pf = pa.tile([1, BPC], f32, tag="A", name=f"pf{rep}")
    nc.tensor.matmul(pf[:], lhsT=ones[:], rhs=rall[:], start=True, stop=True)
    ob = finp.tile([1, BPC], f32, tag="ob", name=f"ob{rep}")
    nc.scalar.mul(ob[:], pf[:], 1.0 / NQ)
    nc.sync.dma_start(out=o_d[:], in_=ob[:])


def _build(loop_reps=None, parts=31):
    import concourse.bacc as bacc
    import concourse.mybir as mybir
    import concourse.tile as tile

    f32 = mybir.dt.float32
    bf16 = mybir.dt.bfloat16

    nc = bacc.Bacc("TRN2", target_bir_lowering=False, debug=False)
    q_d = nc.dram_tensor("q", [BPC, D, NQ], bf16, kind="ExternalInput").ap()
    s_d = nc.dram_tensor("s", [BPC, D, NS], bf16, kind="ExternalInput").ap()
    o_d = nc.dram_tensor("o", [1, BPC], f32, kind="ExternalOutput").ap()

    with tile.TileContext(nc) as tc:
        with (
            tc.tile_pool(name="qp", bufs=3) as qp,
            tc.tile_pool(name="sp", bufs=3) as sp,
            tc.tile_pool(
                name="pa",
                bufs=2 if SCAN_MODE else (1 if PAIR_A else 2),
                space="PSUM",
            ) as pa,
            tc.tile_pool(name="pb", bufs=2, space="PSUM") as pb,
            tc.tile_pool(name="bcp", bufs=3) as bcp,
            tc.tile_pool(name="tree", bufs=2) as trp,
            tc.tile_pool(name="rp", bufs=2) as rp,
            tc.tile_pool(name="fin", bufs=1) as finp,
        ):
            ones = finp.tile([128, 1], f32, tag="ones")
            nc.vector.memset(ones[:], 1.0)
            rall = finp.tile([128, BPC], f32, tag="rall")
            pools = (qp, sp, pa, pb, bcp, trp, rp, finp)

            if loop_reps is None:
                _emit_body(nc, mybir, q_d, s_d, o_d, ones, rall, pools, parts=parts)
            else:
                with tc.For_i(0, loop_reps, 1):
                    _emit_body(
                        nc, mybir, q_d, s_d, o_d, ones, rall, pools, parts=parts
                    )

    nc.compile()
    return nc


def _to_bf16(x):
    import ml_dtypes

    return np.ascontiguousarray(x, dtype=np.float32).astype(ml_dtypes.bfloat16)


def _prep_inputs(query_local, support_local):
    q = _to_bf16(query_local).reshape(N_CORES, BPC, D, NQ)
    s = _to_bf16(support_local).reshape(N_CORES, BPC, D, NS)
    return [{"q": q[c], "s": s[c]} for c in range(N_CORES)]


def kernel(query_local, support_local):
    from concourse.bass_utils import run_bass_kernel_spmd

    if "nc" not in _cache:
        _cache["nc"] = _build()
    nc = _cache["nc"]

    in_maps = _prep_inputs(query_local, support_local)
    res = run_bass_kernel_spmd(nc, in_maps, list(range(N_CORES)))
    outs = [np.asarray(res.results[c]["o"]).reshape(BPC) for c in range(N_CORES)]
    return np.concatenate(outs, axis=0)
